# revision 22
# baseline (speedup 1.0000x reference)
"""NT-Xent (SimCLR) contrastive loss on 8 Trainium2 NeuronCores.

Strategy (symmetric row/column-sharded similarity matrix):
  Z = concat(z_i, z_j) -> [N=8192, D=256].  exp(sim/T) is symmetric, so the
  full matrix never needs computing: core c owns slab c (rows c*1024 ..
  c*1024+1023) and computes only the [1024, 5120] strip against column
  blocks {c, c+1, c+2, c+3, c+4} (mod 8).  Host-side marshaling ships each
  core a rotated [5120, 256] slice so the program is uniform SPMD.

  Per block (a,b): distance 0 (diag) and 4 appear in both cores' strips
  (distance 4: core a computes (a,a+4), core a+4 computes (a+4,a)), so row
  sums come straight from the strip.  Distance 1-3 blocks appear only once;
  their transposed contribution is recovered as COLUMN sums of exp via
  ones-vector matmuls on the tensor engine, shipped back, and scattered by
  the host into the mirrored rows' totals.

  On device, each core:
  - loads its 5 raw row groups (sync HWDGE), L2-normalizes them (squares on
    ScalarE for the first two groups / DVE for the rest, bit-trick rsqrt +
    fp8e4m3 scale-cast on DVE),
  - stages normalized fp8 rows to DRAM (GpSimd SWDGE) and reads them back
    through the DMA xbar transpose (sync HWDGE, bf16-typed so fp8 d-pairs
    (2p, 2p+1) land byte-interleaved on partition p),
  - computes its [1024, 5120] strip of logits with DoubleRowSwInterleave
    fp8 matmuls (full K=256 per instruction, 2x PE rate; the mode's
    reversed weight-column order just flips row order within each 128-row
    tile, undone on the host), exp(2x) on ScalarE with fused per-partition
    row-sum accumulation,
  - column-sums the exp tiles of distance 1-3 blocks with bf16 ones-matmuls
    interleaved into the following sweep's matmul stream,
  - DMAs out [128, 8] row sums and [1, 3072] column sums.
  Host combines row + mirrored column sums, then computes
  loss = mean(log(total - e^2) - pos/T) in f64 (positive-pair dot products
  are O(N*D) marshaling-side work, as is the final log/sum).
"""

import math

import numpy as np

import concourse.bacc as bacc
import concourse.bass as bass
import concourse.mybir as mybir
import concourse.tile as tile
from concourse.bass_utils import run_bass_kernel_spmd

B, D = 4096, 256
N = 2 * B                      # 8192 rows of Z
N_CORES = 8
SLAB = N // N_CORES            # 1024 rows per core
TEMPERATURE = 0.5
INV_T = 1.0 / TEMPERATURE      # 2.0

F32 = mybir.dt.float32
BF16 = mybir.dt.bfloat16
FP8 = mybir.dt.float8e4
I32 = mybir.dt.int32
ALU = mybir.AluOpType
ACT = mybir.ActivationFunctionType
PERF = mybir.MatmulPerfMode

USE_FP8 = True                 # fp8e4m3 DoubleRowSwInterleave matmuls

COLG = 5                       # column groups per core (own + 4 following)
COLS = COLG * SLAB             # 5120 columns in this core's strip
SUBT = SLAB // 128             # 8 subtiles per group
KT = D // 128                  # 2 contraction tiles (bf16 fallback path)
CHUNK = 512                    # matmul moving free dim
MT = SLAB // 128               # 8 output row tiles per core
# (col_offset, width) of each PSUM/activation sweep; diag group first so the
# first sweep needs only ztn[0] and the scalar engine starts ASAP.
JGS = [(0, 1024), (1024, 2048), (3072, 2048)]
CS_LO, CS_HI = SLAB, 4 * SLAB  # column sums needed for groups 1..3
CS_COLS = CS_HI - CS_LO        # 3072

RSQRT_MAGIC = 0x5F3759DF


def build_program():
    nc = bacc.Bacc(
        "TRN2",
        target_bir_lowering=False,
        debug=False,
        num_devices=N_CORES,
    )
    z_cols = nc.declare_dram_parameter("z_cols", [COLS, D], F32, isOutput=False)
    rowsums = nc.declare_dram_parameter("rowsums", [128, MT], F32, isOutput=True)
    # Two half-partials per column (rows m0-3 and m4-7); host adds them.
    colsums = nc.declare_dram_parameter(
        "colsums", [1, 2 * CS_COLS], F32, isOutput=True)

    zc_t = z_cols.rearrange("(n p) d -> p n d", p=128)  # [128, 40, 256]
    zdt = FP8 if USE_FP8 else BF16

    with tile.TileContext(nc) as tc:
        with (
            tc.tile_pool(name="raw", bufs=3) as rawp,
            tc.tile_pool(name="zn", bufs=2) as znp,
            tc.tile_pool(name="small", bufs=2) as small,
            tc.tile_pool(name="zt", bufs=1) as ztp,
            tc.tile_pool(name="ex", bufs=2) as exp_,
            tc.tile_pool(name="psum", bufs=2, space="PSUM") as psum_pool,
            tc.tile_pool(name="dram", bufs=1, space="DRAM") as dram,
        ):
            # Warm the Exp activation table while DMAs run.
            warm = small.tile([128, 1], F32, tag="warm")
            nc.vector.memset(warm[:], 0.0)
            nc.scalar.activation(warm[:], warm[:], ACT.Exp)
            # All-ones SwInterleave lhsT for fp8 column sums ([128, 256] so
            # num_active_cols = 128; every output row repeats the column sum).
            ones2 = small.tile([128, 2 * 128], FP8, tag="ones2")
            nc.vector.memset(ones2[:], 1.0)

            # Persistent transposed normalized embeddings, one per group.
            if USE_FP8:
                ztn = [
                    ztp.tile([128, 2 * SLAB], FP8, tag=f"ztn{g}", name=f"ztn{g}")
                    for g in range(COLG)
                ]
                # [128, 2, SLAB] views: dim1 = fp8 byte within the d-pair.
                zvs = [z[:].rearrange("p (j two) -> p two j", two=2) for z in ztn]
            else:
                ztn = [
                    ztp.tile([128, KT, SLAB], BF16, tag=f"ztn{g}", name=f"ztn{g}")
                    for g in range(COLG)
                ]

            # All raw loads issue up-front on the sync HWDGE ring so no load
            # queues behind a transpose that is still waiting on its store.
            raws = []
            for g in range(COLG):
                raw = rawp.tile([128, SUBT, D], F32, tag=f"raw{g % 3}")
                if g < 2:
                    # Halved loads: the first squares start one half sooner.
                    h = SUBT // 2
                    nc.sync.dma_start(
                        raw[:, :h], zc_t[:, g * SUBT : g * SUBT + h])
                    nc.sync.dma_start(
                        raw[:, h:], zc_t[:, g * SUBT + h : (g + 1) * SUBT])
                else:
                    nc.sync.dma_start(
                        raw[:], zc_t[:, g * SUBT : (g + 1) * SUBT])
                raws.append(raw)

            def squares_scalar(g):
                """Row sums-of-squares on ScalarE (idle during phase A)."""
                sqd = znp.tile([128, D], BF16, tag="sqd_s", name="sqd")
                n2 = small.tile([128, SUBT], F32, tag="n2", name="n2")
                for t in range(SUBT):
                    nc.scalar.activation(
                        sqd[:], raws[g][:, t], ACT.Square,
                        accum_out=n2[:, t : t + 1])
                return n2

            def squares_dve(g):
                sqd = znp.tile([128, D], F32, tag="sqd_v", name="sqd")
                n2 = small.tile([128, SUBT], F32, tag="n2", name="n2")
                for t in range(SUBT):
                    nc.vector.scalar_tensor_tensor(
                        sqd[:], raws[g][:, t], 1.0, raws[g][:, t],
                        op0=ALU.bypass, op1=ALU.mult,
                        accum_out=n2[:, t : t + 1])
                return n2

            def rsqrt(n2):
                """inv = 1/sqrt(n2) on DVE: quake seed + 2 Newton steps."""
                t_int = small.tile([128, SUBT], I32, tag="rsq_i", name="ri")
                y = small.tile([128, SUBT], F32, tag="rsq_y", name="ry")
                a = small.tile([128, SUBT], F32, tag="rsq_a", name="ra")
                c = small.tile([128, SUBT], F32, tag="rsq_c", name="rc")
                inv = small.tile([128, SUBT], F32, tag="inv", name="inv")
                nc.vector.tensor_scalar(
                    t_int[:], n2[:].bitcast(I32), 1, None,
                    op0=ALU.logical_shift_right)
                nc.vector.tensor_scalar(
                    y[:].bitcast(I32), t_int[:], -1, RSQRT_MAGIC,
                    op0=ALU.mult, op1=ALU.add)
                for it in range(2):
                    nc.vector.scalar_tensor_tensor(
                        a[:], y[:], 1.0, y[:], op0=ALU.bypass, op1=ALU.mult)
                    nc.vector.scalar_tensor_tensor(
                        c[:], a[:], -0.5, n2[:], op0=ALU.mult, op1=ALU.mult)
                    nc.vector.scalar_tensor_tensor(
                        inv[:] if it == 1 else y[:], c[:], 1.5, y[:],
                        op0=ALU.add, op1=ALU.mult)
                return inv

            def cast_store_transpose(g, inv):
                """DVE scale-cast to fp8/bf16, stage to DRAM, xbar-transpose
                back into ztn[g]."""
                zn = znp.tile([128, SUBT, D], zdt, tag="zn", name="zn")
                for t in range(SUBT):
                    nc.vector.tensor_scalar(
                        zn[:, t], raws[g][:, t], inv[:, t : t + 1], None,
                        op0=ALU.mult)
                zn_dram = dram.tile(
                    [SLAB, D], zdt, tag=f"zn_dram{g}", name=f"zn_dram{g}")
                # Stage off the sync ring (a store waiting on its cast would
                # head-of-line-block the transposes there, ~15us).  g0/g1 use
                # the scalar HWDGE ring -- they issue before the main ACT
                # stream begins.  g2+ would block ACTs behind them, so they
                # take the GpSimd SWDGE queue; with only three entries it
                # reaches g2 right after its cast (a 5-deep GpSimd queue
                # paced stores ~7.5us apart and starved the g2 transpose).
                store_eng = nc.scalar if g < 2 else nc.gpsimd
                store_eng.dma_start(
                    zn_dram[:].rearrange("(n p) d -> p n d", p=128), zn[:])
                if USE_FP8:
                    # bf16-typed transpose moves fp8 d-pairs (2p, 2p+1) as one
                    # unit onto partition p: one transpose per group.
                    nc.sync.dma_start(
                        out=ztn[g][:].bitcast(BF16),
                        in_=zn_dram[:].bitcast(BF16),
                        transpose=True)
                else:
                    for k in range(KT):
                        nc.sync.dma_start(
                            out=ztn[g][:, k, :],
                            in_=zn_dram[:, k * 128 : (k + 1) * 128],
                            transpose=True)

            # Phase A: ScalarE covers g0/g1 squares so DVE reaches the later
            # groups sooner; DVE emission order prioritizes what the matmul
            # stream needs first (ztn0, then g1/g2 for sweep 1, ...).
            n2_0 = squares_scalar(0)
            n2_1 = squares_scalar(1)
            n2_2 = squares_scalar(2)
            # Re-warm Exp in case Square lives in a different table set.
            nc.scalar.activation(warm[:], warm[:], ACT.Exp)
            # DVE order tracks what the matmul stream needs next: ztn0 gates
            # the whole stream, so its cast comes before anything for g2+.
            inv0 = rsqrt(n2_0)
            cast_store_transpose(0, inv0)
            inv1 = rsqrt(n2_1)
            cast_store_transpose(1, inv1)
            inv2 = rsqrt(n2_2)
            cast_store_transpose(2, inv2)
            for g in (3, 4):
                n2_g = squares_dve(g)
                inv_g = rsqrt(n2_g)
                cast_store_transpose(g, inv_g)

            def emit_smm(ps, m, col0, width):
                """Similarity matmuls for one [128, width] PSUM tile."""
                if USE_FP8:
                    lhsT = ztn[0][:, m * 2 * 128 : (m + 1) * 2 * 128]
                    for c in range(width // CHUNK):
                        g, off = divmod(col0 + c * CHUNK, SLAB)
                        nc.tensor.matmul(
                            ps[:, c * CHUNK : (c + 1) * CHUNK],
                            lhsT, zvs[g][:, :, off : off + CHUNK],
                            start=True, stop=True,
                            perf_mode=PERF.DoubleRowSwInterleave)
                else:
                    for k in range(KT):
                        for c in range(width // CHUNK):
                            g, off = divmod(col0 + c * CHUNK, SLAB)
                            nc.tensor.matmul(
                                ps[:, c * CHUNK : (c + 1) * CHUNK],
                                ztn[0][:, k, m * 128 : (m + 1) * 128],
                                ztn[g][:, k, off : off + CHUNK],
                                start=(k == 0), stop=(k == KT - 1))

            def emit_colsum(ex_tiles, jg_col0, col, half):
                """Half-partial column sums of one 512-col chunk (row tiles
                m0-3 for half 0, m4-7 for half 1).

                ex tiles are fp8 [128, 2, width] m-pairs, so each SwInterleave
                ones-matmul sums 256 rows at 2x rate (2 matmuls per half).
                Halves let the ones-matmuls spread across twice as many
                interleave slots of the following sweep's matmul stream."""
                cs = psum_pool.tile([128, CHUNK], F32, tag="ps", name="cs")
                local = col - jg_col0
                for i, mp in enumerate((2 * half, 2 * half + 1)):
                    nc.tensor.matmul(
                        cs[:], ones2[:],
                        ex_tiles[mp][:, :, local : local + CHUNK],
                        start=(i == 0), stop=(i == 1),
                        perf_mode=PERF.DoubleRowSwInterleave)
                out_off = half * CS_COLS + col - CS_LO
                nc.vector.tensor_copy(
                    colsum_sb[:, out_off : out_off + CHUNK], cs[0:1, :])

            # Main pass: strip logits, exp, fused row sums, column sums.
            rsparts = small.tile(
                [128, MT, len(JGS)], F32, tag="rsparts", name="rsparts")
            colsum_sb = small.tile(
                [1, 2 * CS_COLS], F32, tag="colsum_sb", name="colsum_sb")
            ex_sets = []
            for jg, (col0, width) in enumerate(JGS):
                ex_tiles = []
                for m in range(MT):
                    ps = psum_pool.tile([128, width], F32, tag="ps", name="ps")
                    emit_smm(ps, m, col0, width)
                    if m % 2 == 0:
                        ex = exp_.tile(
                            [128, 2, width], FP8, tag=f"ex{m // 2}", name="ex")
                        ex_tiles.append(ex)
                    nc.scalar.activation(
                        ex_tiles[m // 2][:, m % 2], ps[:], ACT.Exp,
                        scale=INV_T,
                        accum_out=rsparts[:, m, jg : jg + 1])
                    # Interleave column-sum half-partials into the matmul
                    # stream as soon as their ex m-pairs exist: sweep 1's
                    # first halves run inside sweep 1 itself (m4-7), its
                    # second halves + sweep 2's own inside sweep 2.
                    pc0, _ = JGS[1]
                    if jg == 1 and m >= 4:
                        emit_colsum(ex_tiles, pc0, pc0 + (m - 4) * CHUNK, 0)
                    elif jg == 2 and m < 4:
                        emit_colsum(ex_sets[1], pc0, pc0 + m * CHUNK, 1)
                    elif jg == 2 and m >= 6:
                        emit_colsum(
                            ex_tiles, col0, col0 + (m - 6) * CHUNK, 0)
                ex_sets.append(ex_tiles)
            # Second halves of sweep 2's two chunks as the (short) tail.
            for c in range(2):
                emit_colsum(ex_sets[2], JGS[2][0], JGS[2][0] + c * CHUNK, 1)

            rs = small.tile([128, MT], F32, tag="rs", name="rs")
            nc.vector.tensor_reduce(
                rs[:].rearrange("p (m o) -> p m o", o=1), rsparts[:],
                axis=mybir.AxisListType.X, op=ALU.add,
            )
            nc.sync.dma_start(rowsums[:], rs[:])
            nc.sync.dma_start(colsums[:], colsum_sb[:])
    nc.compile()
    return nc


_PROGRAM = None


def _get_program():
    global _PROGRAM
    if _PROGRAM is None:
        _PROGRAM = build_program()
    return _PROGRAM


def run_device(z_i, z_j, **spmd_kwargs):
    """Run the SPMD kernel; returns ([N] row sums of exp(sim/T), raw results)."""
    nc = _get_program()
    z_all = np.concatenate([z_i, z_j], axis=0)
    z_wrap = np.concatenate([z_all, z_all[: COLS - SLAB]], axis=0)
    in_maps = [
        {"z_cols": np.ascontiguousarray(z_wrap[c * SLAB : c * SLAB + COLS])}
        for c in range(N_CORES)
    ]
    out = run_bass_kernel_spmd(nc, in_maps, list(range(N_CORES)), **spmd_kwargs)
    total = np.zeros(N, dtype=np.float64)
    idx = np.arange(CS_COLS)
    for c, r in enumerate(out.results):
        rows = np.asarray(r["rowsums"]).astype(np.float64).T  # [MT, 128]
        if USE_FP8:
            # SwInterleave weight-column reversal: partition p = row 127-p.
            rows = rows[:, ::-1]
        total[c * SLAB : (c + 1) * SLAB] += rows.reshape(SLAB)
        cols = (np.asarray(r["colsums"]).astype(np.float64)
                .reshape(2, CS_COLS).sum(axis=0))
        np.add.at(total, ((c + 1) * SLAB + idx) % N, cols)
    return total, out


def finalize(z_i, z_j, rowsums):
    """Host-side O(N) finish: diagonal removal, log, positive-pair term."""
    rs = rowsums.astype(np.float64)
    lse = np.log(rs - math.exp(INV_T))          # drop masked diagonal exp(1/T)
    zi = z_i.astype(np.float64)
    zj = z_j.astype(np.float64)
    zi /= np.linalg.norm(zi, axis=1, keepdims=True)
    zj /= np.linalg.norm(zj, axis=1, keepdims=True)
    pos = np.sum(zi * zj)                       # = 0.5 * sum_r pos_r
    loss = (lse.sum() - 2.0 * pos * INV_T) / N
    return np.asarray(loss, dtype=np.float32)


def kernel(z_i, z_j):
    z_i = np.ascontiguousarray(np.asarray(z_i, dtype=np.float32))
    z_j = np.ascontiguousarray(np.asarray(z_j, dtype=np.float32))
    rowsums, _ = run_device(z_i, z_j)
    return finalize(z_i, z_j, rowsums)


if __name__ == "__main__":
    rng = np.random.default_rng(0)
    a = rng.standard_normal((B, D), dtype=np.float32)
    b = rng.standard_normal((B, D), dtype=np.float32)
    print(kernel(a, b))
